# revision 1
# baseline (speedup 1.0000x reference)
"""Trainium2 Bass kernel for nn_CorrelationHead (8-core SPMD, data parallel over B).

Math reformulation (validated ~1e-6 vs the jax reference in fp32):
  corr[b,p,q,i,j] = sum_c patch1[b,c,i,j] * patch2[b,c, i+2p-20, j+2q-20]
  out[b,n] = sum w[n,:]*corr[b,:] + bias[n]
           = sum_{ij,yx} (P1[b]^T P2[b])[ij,yx] * W3[n,ij,yx] + bias[n]
  where W3 gathers w_bbox onto the 49x49 (ij,yx) grid (displacements that
  land outside the 7x7 patch hit zero padding and drop out).

Device mapping per core (64 samples), bf16, raw bass (hand-rolled sems):
  - host packs both patches channel-pair-interleaved: Y[b,p,196] =
    [p1[b,2p] | p2[b,2p] | p1[b,2p+1] | p2[b,2p+1]]  -> 392B-contiguous
    DMA descriptors; loaded as 16 sliced DMAs alternating the two HWDGE
    rings so the PE can chase the stream.
  - stage 1: per sample two accumulating K=64 matmuls (even/odd channel
    halves) -> PSUM A^T[b] [yx=49, ij=49]; 4 samples per PSUM slot-group,
    batch-cast (f32->bf16) to SBUF acat[yx,b,ij], alternating DVE/ACT.
  - stage 2: 49 accumulating matmuls contract ij (lhsT = 4 columns of the
    gathered weights, rhs = acat[:,:,ij]), interleaved over 4 independent
    accumulation chains pinned to distinct PE column strips via
    tile_position=(0,32c) so each strip's weight buffer loads while other
    strips compute; chain partials land at partitions 32c..32c+3.
  - a final selector matmul (0/1 matrix, with the bias folded in via a
    DMA'd ones-row at partition 127) sums the 4 chains across partitions.
  - patches are host-packed 4-samples-per-partition-row (1568B DMA runs)
    and streamed as 16 slices round-robined over three issue queues
    (sync/scalar HWDGE + gpsimd SWDGE); small loads go last on gpsimd.
"""

import numpy as np

import concourse.bass as bass
import concourse.mybir as mybir
from concourse import bacc
from concourse.bass_utils import run_bass_kernel_spmd

N_CORES = 8
B, C, HW = 512, 128, 49
BS = B // N_CORES   # 64 samples per core
CP = C // 2         # 64 partitions, 2 channels each
FW = 4 * HW         # 196 packed row: p1/even, p2/even, p1/odd, p2/odd
PAD = 20
GROUPS = 4
GB = BS // GROUPS   # 16
PSB = 4             # samples per PSUM slot-group
NCHAIN = 4

_F32 = mybir.dt.float32
_BF16 = mybir.dt.bfloat16


def _build_wst2(w_bbox: np.ndarray) -> np.ndarray:
    W3 = np.zeros((4, 49, 49), np.float32)
    for i in range(7):
        for j in range(7):
            for y in range(7):
                for x in range(7):
                    if (y - i) % 2 == 0 and (x - j) % 2 == 0:
                        p = (y - i + PAD) // 2
                        q = (x - j + PAD) // 2
                        W3[:, i * 7 + j, y * 7 + x] = w_bbox[
                            :, ((p * 21 + q) * 7 + i) * 7 + j
                        ]
    return np.ascontiguousarray(W3.transpose(2, 1, 0).reshape(49, 196))


def build_nc() -> bass.Bass:
    nc = bacc.Bacc("TRN2", target_bir_lowering=False, debug=False)
    pp = nc.dram_tensor("pp", [BS // 4, CP, 4 * FW], _BF16, kind="ExternalInput")
    wst2 = nc.dram_tensor("wst2", [49, 196], _BF16, kind="ExternalInput")
    seld = nc.dram_tensor("seld", [128, 4], _BF16, kind="ExternalInput")
    onesd = nc.dram_tensor("onesd", [1, BS], _BF16, kind="ExternalInput")
    out = nc.dram_tensor("out", [4, BS], _F32, kind="ExternalOutput")

    ppr = pp[:].rearrange("b p f -> p b f")

    from contextlib import ExitStack

    with ExitStack() as ctx:
        ts_ = [
            ctx.enter_context(nc.sbuf_tensor(f"t{g}", [CP, GB // 4, 4 * FW], _BF16))
            for g in range(GROUPS)
        ]
        t0, t1, t2, t3 = ts_
        acat = ctx.enter_context(nc.sbuf_tensor("acat", [49, BS, HW], _BF16))
        w_t = ctx.enter_context(nc.sbuf_tensor("w_t", [49, 196], _BF16))
        sel_w = ctx.enter_context(nc.sbuf_tensor("sel_w", [128, 4], _BF16))
        sel_sb = ctx.enter_context(nc.sbuf_tensor("sel_sb", [128, BS], _BF16))
        out_sb = ctx.enter_context(nc.sbuf_tensor("out_sb", [4, BS], _F32))
        ps = ctx.enter_context(nc.psum_tensor("ps", [128, 8, 512], _F32))
        (sW1, sW2, sMM, sCastD, sCastA, sS2, sOut, sDone,
         sW3, sSel, sS3) = (
            ctx.enter_context(nc.semaphore(nm))
            for nm in (
                "sW1", "sW2",
                "sMM", "sCastD", "sCastA", "sS2", "sOut", "sDone",
                "sW3", "sSel", "sS3",
            )
        )
        sD = [
            ctx.enter_context(nc.semaphore(f"sD{i}")) for i in range(16)
        ]
        sS2c = [
            ctx.enter_context(nc.semaphore(f"sS2c{i}")) for i in range(NCHAIN)
        ]
        block = ctx.enter_context(nc.Block())

        @block.sync
        def _(sync):
            for sl in range(0, 16, 3):
                sync.dma_start(
                    out=ts_[sl // 4][:, sl % 4, :],
                    in_=ppr[:, sl, :],
                ).then_inc(sD[sl], 16)
            sync.wait_ge(sOut, 1)
            sync.dma_start(out=out[:], in_=out_sb[:]).then_inc(sDone, 16)
            sync.wait_ge(sDone, 16)

        @block.scalar
        def _(scalar):
            for sl in range(1, 16, 3):
                scalar.dma_start(
                    out=ts_[sl // 4][:, sl % 4, :],
                    in_=ppr[:, sl, :],
                ).then_inc(sD[sl], 16)
            for T in range(1, 16, 2):  # odd slot-groups cast on ACT
                scalar.wait_ge(sMM, T + 1)
                nc.scalar.copy(
                    acat[:, T * PSB : (T + 1) * PSB, :],
                    ps[0:49, (T % 2) * 4 : (T % 2) * 4 + 4, 0:HW],
                ).then_inc(sCastA, 1)

        @block.gpsimd
        def _(gpsimd):
            for sl in range(2, 16, 3):
                gpsimd.dma_start(
                    out=ts_[sl // 4][:, sl % 4, :],
                    in_=ppr[:, sl, :],
                ).then_inc(sD[sl], 16)
            gpsimd.dma_start(out=w_t[:], in_=wst2[:]).then_inc(sW1, 16)
            gpsimd.dma_start(out=sel_w[:], in_=seld[:]).then_inc(sW3, 16)
            gpsimd.dma_start(out=sel_sb[127:128, :], in_=onesd[:]).then_inc(
                sW2, 16
            )

        @block.tensor
        def _(tensor):
            for g in range(GROUPS):
                t = ts_[g]
                for k in range(GB):
                    bb = g * GB + k
                    T, j = bb // PSB, bb % PSB
                    if j == 0:
                        tensor.wait_ge(sD[T], 16)
                    if j == 0 and T >= 2:
                        # reuse of PSUM slot T%2: wait for cast of tile T-2
                        if T % 2 == 0:
                            tensor.wait_ge(sCastD, (T - 2) // 2 + 1)
                        else:
                            tensor.wait_ge(sCastA, (T - 2) // 2 + 1)
                    slot = T % 2
                    q, base = k // 4, (k % 4) * FW
                    nc.tensor.matmul(
                        ps[0:49, slot * 4 + j, 0:HW],
                        t[:, q, base + 49 : base + 98],
                        t[:, q, base : base + 49],
                        start=True,
                        stop=False,
                    )
                    mm2 = nc.tensor.matmul(
                        ps[0:49, slot * 4 + j, 0:HW],
                        t[:, q, base + 147 : base + 196],
                        t[:, q, base + 98 : base + 147],
                        start=False,
                        stop=True,
                    )
                    if j == PSB - 1:
                        mm2.then_inc(sMM, 1)
            # stage 2: needs all of acat + w_t
            tensor.wait_ge(sCastD, 8)
            tensor.wait_ge(sCastA, 8)
            tensor.wait_ge(sW1, 16)
            for ij in range(HW):
                c = ij % NCHAIN
                mm = nc.tensor.matmul(
                    ps[32 * c : 32 * c + 4, c, 0:BS],
                    w_t[:, ij * 4 : (ij + 1) * 4],
                    acat[:, :, ij],
                    start=(ij < NCHAIN),
                    stop=(ij + NCHAIN >= HW),
                    tile_position=(0, 32 * c),
                )
                if ij + NCHAIN >= HW:
                    mm.then_inc(sS2c[c], 1)
            tensor.wait_ge(sSel, 1)
            tensor.wait_ge(sW3, 16)
            tensor.wait_ge(sW2, 16)
            nc.tensor.matmul(
                ps[0:4, 7, 0:BS], sel_w[:], sel_sb[:], start=True, stop=True
            ).then_inc(sS3, 1)

        @block.vector
        def _(vector):
            nc.vector.memset(sel_sb[0:127, :], 0.0)
            for T in range(0, 16, 2):  # even slot-groups cast on DVE
                vector.wait_ge(sMM, T + 1)
                nc.vector.tensor_copy(
                    acat[:, T * PSB : (T + 1) * PSB, :],
                    ps[0:49, (T % 2) * 4 : (T % 2) * 4 + 4, 0:HW],
                ).then_inc(sCastD, 1)
            last_cast = None
            for c in (1, 2, 3, 0):  # chain completion order
                vector.wait_ge(sS2c[c], 1)
                last_cast = nc.vector.tensor_copy(
                    sel_sb[32 * c : 32 * c + 4, :],
                    ps[32 * c : 32 * c + 4, c, 0:BS],
                )
            last_cast.then_inc(sSel, 1)
            vector.wait_ge(sS3, 1)
            nc.vector.tensor_copy(out_sb[:], ps[0:4, 7, 0:BS]).then_inc(sOut, 1)

    nc.compile()
    return nc


def _prep_inputs(inputs):
    import ml_dtypes

    p1 = np.asarray(inputs["patch1"], np.float32).reshape(B, C, HW)
    p2 = np.asarray(inputs["patch2"], np.float32).reshape(B, C, HW)
    bf = ml_dtypes.bfloat16
    Y = np.empty((B, CP, FW), bf)
    Y[:, :, 0:49] = p1[:, 0::2, :]
    Y[:, :, 49:98] = p2[:, 0::2, :]
    Y[:, :, 98:147] = p1[:, 1::2, :]
    Y[:, :, 147:196] = p2[:, 1::2, :]
    # pack 4 consecutive samples along each partition row -> 1568B runs
    Y4 = np.ascontiguousarray(
        Y.reshape(B // 4, 4, CP, FW).transpose(0, 2, 1, 3).reshape(
            B // 4, CP, 4 * FW
        )
    )
    wst2 = _build_wst2(np.asarray(inputs["w_bbox"], np.float32)).astype(bf)
    seld = np.zeros((128, 4), bf)
    for c in range(NCHAIN):
        for n in range(4):
            seld[32 * c + n, n] = 1
    seld[127, :] = np.asarray(inputs["b_bbox"], np.float32).astype(bf)
    in_maps = []
    for c in range(N_CORES):
        sl = slice(c * (BS // 4), (c + 1) * (BS // 4))
        in_maps.append(
            {
                "pp": np.ascontiguousarray(Y4[sl]),
                "wst2": wst2,
                "seld": seld,
                "onesd": np.ones((1, BS), bf),
            }
        )
    return in_maps


def _run(inputs, trace: bool = False):
    nc = build_nc()
    in_maps = _prep_inputs(inputs)
    res = run_bass_kernel_spmd(
        nc, in_maps, core_ids=list(range(N_CORES)), trace=trace
    )
    out = np.concatenate(
        [res.results[c]["out"].T for c in range(N_CORES)], axis=0
    ).astype(np.float32)
    return out, res


def kernel(**inputs) -> np.ndarray:
    out, _ = _run(inputs, trace=False)
    return out



# revision 2
# speedup vs baseline: 1.0714x; 1.0714x over previous
"""Trainium2 Bass kernel for nn_CorrelationHead (8-core SPMD, data parallel over B).

Math: out[b,n] = sum_{yx,ij} (P2[b]^T P1[b])[yx,ij] * W3[yx,ij,n] + bias[n]

v5 (trace-driven):
  - Input: 5 chunk DMAs (16,16,16,8,8 samples) all on the sync HWDGE ring:
    one ring row keeps all 16 SDMA engines on one FIFO stream (v3 showed
    two rows halve effective bandwidth for the early chunks). Small last
    chunks cut the final sem-lag exposure.
  - Stage 1: one K=128 matmul per sample, alternating PE column strips
    (even samples -> psum partitions 0:49, odd -> 64:113) so LDWEIGHTS
    overlaps MATMUL. 8 samples per PSUM bank; all 64 fit -> no reuse sems.
  - Casts: evens on DVE, odds on ACT into acat2[128, 64, 49] (columns 0:32
    even samples rows 0:49 live, 32:64 odd rows 64:113 live; garbage
    regions zeroed once by gpsimd).
  - Stage 2: 49 K=113 matmuls, duplicated weights, 4 PE column strips.
  - Tail: the 4 chain partials are copied psum->SBUF f32 (DVE+ACT in
    parallel) and DMA'd out as [100, 64]; the 4-way fold + bias + column
    unpermute happen on the host. No final-DMA-completion wait: the
    end-of-block engine drain + fixed ~7.6us teardown dwarf the 26KB DMA.
"""

import numpy as np

import concourse.bass as bass
import concourse.mybir as mybir
from concourse import bacc
from concourse.bass_utils import run_bass_kernel_spmd

N_CORES = 8
B, C, HW = 512, 128, 49
BS = B // N_CORES    # 64 samples per core
CHUNKS = (8, 16, 16, 16, 8)
NG = 8               # psum bank groups
GS = BS // NG        # 8 samples per bank
PAD = 20

_F32 = mybir.dt.float32
_BF16 = mybir.dt.bfloat16


def _build_wst2(w_bbox: np.ndarray) -> np.ndarray:
    W3 = np.zeros((4, 49, 49), np.float32)
    for i in range(7):
        for j in range(7):
            for y in range(7):
                for x in range(7):
                    if (y - i) % 2 == 0 and (x - j) % 2 == 0:
                        p = (y - i + PAD) // 2
                        q = (x - j + PAD) // 2
                        W3[:, i * 7 + j, y * 7 + x] = w_bbox[
                            :, ((p * 21 + q) * 7 + i) * 7 + j
                        ]
    return np.ascontiguousarray(W3.transpose(2, 1, 0).reshape(49, 196))


def build_nc() -> bass.Bass:
    nc = bacc.Bacc("TRN2", target_bir_lowering=False, debug=False)
    pp = nc.dram_tensor("pp", [C, BS, 98], _BF16, kind="ExternalInput")
    wc = nc.dram_tensor("wc", [113, 196], _BF16, kind="ExternalInput")
    out = nc.dram_tensor("out", [100, BS], _F32, kind="ExternalOutput")

    from contextlib import ExitStack

    with ExitStack() as ctx:
        x_sb = ctx.enter_context(nc.sbuf_tensor("x_sb", [C, BS, 98], _BF16))
        acat2 = ctx.enter_context(nc.sbuf_tensor("acat2", [128, BS, HW], _BF16))
        wsb = ctx.enter_context(nc.sbuf_tensor("wsb", [113, 196], _BF16))
        outp = ctx.enter_context(nc.sbuf_tensor("outp", [100, BS], _F32))
        ps = ctx.enter_context(nc.psum_tensor("ps", [128, 8, 512], _F32))
        (sW, sMM, sCastD, sCastA, sS2, sOutD, sOutA, sDone) = (
            ctx.enter_context(nc.semaphore(nm))
            for nm in ("sW", "sMM", "sCastD", "sCastA", "sS2",
                       "sOutD", "sOutA", "sDone")
        )
        sIn = [ctx.enter_context(nc.semaphore(f"sIn{i}"))
               for i in range(len(CHUNKS))]
        block = ctx.enter_context(nc.Block())

        @block.sync
        def _(sync):
            o = 0
            for g, n in enumerate(CHUNKS):
                sync.dma_start(
                    out=x_sb[:, o : o + n, :], in_=pp[:, o : o + n, :]
                ).then_inc(sIn[g], 16)
                o += n
            sync.wait_ge(sOutD, 1)
            sync.wait_ge(sOutA, 1)
            sync.dma_start(out=out[:], in_=outp[:]).then_inc(sDone, 16)

        @block.scalar
        def _(scalar):
            for g in range(NG):  # odd-parity casts
                scalar.wait_ge(sMM, g + 1)
                nc.scalar.copy(
                    acat2[64:113, 32 + g * 4 : 32 + (g + 1) * 4, :],
                    ps[64:113, g, 196:392],
                ).then_inc(sCastA, 1)
            scalar.wait_ge(sS2, 3)
            nc.scalar.copy(outp[96:100, :], ps[96:100, 3, 0:BS])
            scalar.wait_ge(sS2, 4)
            nc.scalar.copy(outp[0:4, :], ps[0:4, 0, 0:BS]).then_inc(sOutA, 1)

        @block.gpsimd
        def _(gpsimd):
            gpsimd.dma_start(out=wsb[:], in_=wc[:]).then_inc(sW, 16)

        @block.tensor
        def _(tensor):
            bounds = []
            o = 0
            for n in CHUNKS:
                bounds.append(o)
                o += n
            for s in range(BS):
                if s in bounds:
                    tensor.wait_ge(sIn[bounds.index(s)], 16)
                g, idx, par = s // GS, (s % GS) // 2, s % 2
                col = par * 196 + idx * HW
                pbase = 64 * par
                mm = nc.tensor.matmul(
                    ps[pbase : pbase + 49, g, col : col + HW],
                    x_sb[:, s, 49:98],
                    x_sb[:, s, 0:49],
                    start=True,
                    stop=True,
                )
                if s % GS == GS - 1:
                    mm.then_inc(sMM, 1)
            tensor.wait_ge(sCastD, NG)
            tensor.wait_ge(sCastA, NG)
            tensor.wait_ge(sW, 16)
            for ij in range(HW):
                c = ij % 4
                mm = nc.tensor.matmul(
                    ps[32 * c : 32 * c + 4, c, 0:BS],
                    wsb[:, ij * 4 : (ij + 1) * 4],
                    acat2[0:113, :, ij],
                    start=(ij < 4),
                    stop=(ij + 4 >= HW),
                    tile_position=(0, 32 * c),
                )
                if ij + 4 >= HW:
                    mm.then_inc(sS2, 1)

        @block.vector
        def _(vector):
            # zero acat2's garbage regions: odd cols rows 0:64, even cols
            # rows 64:128, and even cols rows 32:64 (49:64 is garbage;
            # 32:49 is rewritten by this engine's own casts afterwards).
            # Ordered vs the casts by same-engine program order; stage 2
            # waits on sCastD>=8 which is later still.
            nc.vector.memset(acat2[0:64, 32:64, :], 0.0)
            nc.vector.memset(acat2[64:128, 0:32, :], 0.0)
            nc.vector.memset(acat2[32:64, 0:32, :], 0.0)
            for g in range(NG):  # even-parity casts
                vector.wait_ge(sMM, g + 1)
                nc.vector.tensor_copy(
                    acat2[0:49, g * 4 : (g + 1) * 4, :],
                    ps[0:49, g, 0:196],
                ).then_inc(sCastD, 1)
            vector.wait_ge(sS2, 1)
            nc.vector.tensor_copy(outp[32:36, :], ps[32:36, 1, 0:BS])
            vector.wait_ge(sS2, 2)
            nc.vector.tensor_copy(outp[64:68, :], ps[64:68, 2, 0:BS]).then_inc(
                sOutD, 1
            )

    nc.compile()
    return nc


def _prep_inputs(inputs):
    import ml_dtypes

    bf = ml_dtypes.bfloat16
    p1 = np.asarray(inputs["patch1"], np.float32).reshape(B, C, HW)
    p2 = np.asarray(inputs["patch2"], np.float32).reshape(B, C, HW)
    X = np.empty((B, C, 98), bf)
    X[:, :, 0:49] = p1
    X[:, :, 49:98] = p2
    wst2 = _build_wst2(np.asarray(inputs["w_bbox"], np.float32)).astype(bf)
    wcv = np.zeros((113, 196), bf)
    wcv[0:49] = wst2
    wcv[64:113] = wst2
    in_maps = []
    for c in range(N_CORES):
        Xc = X[c * BS : (c + 1) * BS]  # [BS, C, 98]
        ppv = np.ascontiguousarray(Xc.transpose(1, 0, 2))  # [C, BS, 98]
        in_maps.append({"pp": ppv, "wc": wcv})
    return in_maps


# device column b' holds sample 2*(b'%32) + b'//32
_COLS = np.arange(BS)
_SAMPLE_OF_COL = 2 * (_COLS % 32) + _COLS // 32


def _run(inputs, trace: bool = False):
    nc = build_nc()
    in_maps = _prep_inputs(inputs)
    bias = np.asarray(inputs["b_bbox"], np.float32)
    res = run_bass_kernel_spmd(
        nc, in_maps, core_ids=list(range(N_CORES)), trace=trace
    )
    parts = []
    for c in range(N_CORES):
        r = res.results[c]["out"]  # [100, BS]
        # fold the 4 chain partials (rows 32c+n) + bias
        folded = (
            r[0:4] + r[32:36] + r[64:68] + r[96:100]
        ).T + bias  # [BS, 4], permuted columns
        oc = np.empty((BS, 4), np.float32)
        oc[_SAMPLE_OF_COL] = folded
        parts.append(oc)
    out = np.concatenate(parts, axis=0).astype(np.float32)
    return out, res


def kernel(**inputs) -> np.ndarray:
    out, _ = _run(inputs, trace=False)
    return out


# revision 3
# speedup vs baseline: 1.1016x; 1.0282x over previous
"""Trainium2 Bass kernel for nn_CorrelationHead (8-core SPMD, data parallel over B).

Math: out[b,n] = sum_{yx,ij} (P2[b]^T P1[b])[yx,ij] * W3[yx,ij,n] + bias[n]

v5 (trace-driven):
  - Input: 5 chunk DMAs (16,16,16,8,8 samples) all on the sync HWDGE ring:
    one ring row keeps all 16 SDMA engines on one FIFO stream (v3 showed
    two rows halve effective bandwidth for the early chunks). Small last
    chunks cut the final sem-lag exposure.
  - Stage 1: one K=128 matmul per sample, alternating PE column strips
    (even samples -> psum partitions 0:49, odd -> 64:113) so LDWEIGHTS
    overlaps MATMUL. 8 samples per PSUM bank; all 64 fit -> no reuse sems.
  - Casts: evens on DVE, odds on ACT into acat2[128, 64, 49] (columns 0:32
    even samples rows 0:49 live, 32:64 odd rows 64:113 live; garbage
    regions zeroed once by gpsimd).
  - Stage 2: 49 K=113 matmuls, duplicated weights, 4 PE column strips.
  - Tail: the 4 chain partials are copied psum->SBUF f32 (DVE+ACT in
    parallel) and DMA'd out as [100, 64]; the 4-way fold + bias + column
    unpermute happen on the host. No final-DMA-completion wait: the
    end-of-block engine drain + fixed ~7.6us teardown dwarf the 26KB DMA.
"""

import numpy as np

import concourse.bass as bass
import concourse.mybir as mybir
from concourse import bacc
from concourse.bass_utils import run_bass_kernel_spmd

N_CORES = 8
B, C, HW = 512, 128, 49
BS = B // N_CORES    # 64 samples per core
CHUNKS = (8, 16, 16, 16, 8)
NG = 8               # psum bank groups
GS = BS // NG        # 8 samples per bank
PAD = 20

_F32 = mybir.dt.float32
_BF16 = mybir.dt.bfloat16


def _build_wst2(w_bbox: np.ndarray) -> np.ndarray:
    W3 = np.zeros((4, 49, 49), np.float32)
    for i in range(7):
        for j in range(7):
            for y in range(7):
                for x in range(7):
                    if (y - i) % 2 == 0 and (x - j) % 2 == 0:
                        p = (y - i + PAD) // 2
                        q = (x - j + PAD) // 2
                        W3[:, i * 7 + j, y * 7 + x] = w_bbox[
                            :, ((p * 21 + q) * 7 + i) * 7 + j
                        ]
    return np.ascontiguousarray(W3.transpose(2, 1, 0).reshape(49, 196))


def build_nc() -> bass.Bass:
    nc = bacc.Bacc("TRN2", target_bir_lowering=False, debug=False)
    pp = nc.dram_tensor("pp", [C, BS, 98], _BF16, kind="ExternalInput")
    wc = nc.dram_tensor("wc", [49, 196], _BF16, kind="ExternalInput")
    out = nc.dram_tensor("out", [100, BS], _F32, kind="ExternalOutput")

    from contextlib import ExitStack

    with ExitStack() as ctx:
        x_sb = ctx.enter_context(nc.sbuf_tensor("x_sb", [C, BS, 98], _BF16))
        acat2 = ctx.enter_context(nc.sbuf_tensor("acat2", [49, BS, HW], _BF16))
        wsb = ctx.enter_context(nc.sbuf_tensor("wsb", [49, 196], _BF16))
        outp = ctx.enter_context(nc.sbuf_tensor("outp", [100, BS], _F32))
        ps = ctx.enter_context(nc.psum_tensor("ps", [128, 8, 512], _F32))
        (sW, sMM, sCastD, sCastA, sS2, sOutD, sOutA, sDone) = (
            ctx.enter_context(nc.semaphore(nm))
            for nm in ("sW", "sMM", "sCastD", "sCastA", "sS2",
                       "sOutD", "sOutA", "sDone")
        )
        sIn = [ctx.enter_context(nc.semaphore(f"sIn{i}"))
               for i in range(len(CHUNKS))]
        block = ctx.enter_context(nc.Block())

        @block.sync
        def _(sync):
            o = 0
            for g, n in enumerate(CHUNKS):
                sync.dma_start(
                    out=x_sb[:, o : o + n, :], in_=pp[:, o : o + n, :]
                ).then_inc(sIn[g], 16)
                o += n
            sync.wait_ge(sOutD, 1)
            sync.wait_ge(sOutA, 1)
            sync.dma_start(out=out[:], in_=outp[:]).then_inc(sDone, 16)

        @block.scalar
        def _(scalar):
            for g in range(NG):  # odd-parity casts
                scalar.wait_ge(sMM, g + 1)
                nc.scalar.copy(
                    acat2[0:49, 32 + g * 4 : 32 + (g + 1) * 4, :],
                    ps[64:113, g, 196:392],
                ).then_inc(sCastA, 1)
            scalar.wait_ge(sS2, 2)
            nc.scalar.copy(outp[64:68, :], ps[64:68, 2, 0:BS])
            scalar.wait_ge(sS2, 3)
            nc.scalar.copy(outp[96:100, :], ps[96:100, 3, 0:BS]).then_inc(
                sOutA, 1
            )

        @block.gpsimd
        def _(gpsimd):
            gpsimd.dma_start(out=wsb[:], in_=wc[:]).then_inc(sW, 16)

        @block.tensor
        def _(tensor):
            bounds = []
            o = 0
            for n in CHUNKS:
                bounds.append(o)
                o += n
            for s in range(BS):
                if s in bounds:
                    tensor.wait_ge(sIn[bounds.index(s)], 16)
                g, idx, par = s // GS, (s % GS) // 2, s % 2
                col = par * 196 + idx * HW
                pbase = 64 * par
                mm = nc.tensor.matmul(
                    ps[pbase : pbase + 49, g, col : col + HW],
                    x_sb[:, s, 49:98],
                    x_sb[:, s, 0:49],
                    start=True,
                    stop=True,
                )
                if s % GS == GS - 1:
                    mm.then_inc(sMM, 1)
            tensor.wait_ge(sCastD, NG)
            tensor.wait_ge(sCastA, NG)
            tensor.wait_ge(sW, 16)
            for ij in range(HW):
                c = ij % 4
                mm = nc.tensor.matmul(
                    ps[32 * c : 32 * c + 4, c, 0:BS],
                    wsb[:, ij * 4 : (ij + 1) * 4],
                    acat2[:, :, ij],
                    start=(ij < 4),
                    stop=(ij + 4 >= HW),
                    tile_position=(0, 32 * c),
                )
                if ij + 4 >= HW:
                    mm.then_inc(sS2, 1)

        @block.vector
        def _(vector):
            for g in range(NG):  # even-parity casts
                vector.wait_ge(sMM, g + 1)
                nc.vector.tensor_copy(
                    acat2[0:49, g * 4 : (g + 1) * 4, :],
                    ps[0:49, g, 0:196],
                ).then_inc(sCastD, 1)
            vector.wait_ge(sS2, 1)
            nc.vector.tensor_copy(outp[32:36, :], ps[32:36, 1, 0:BS])
            vector.wait_ge(sS2, 4)
            nc.vector.tensor_copy(outp[0:4, :], ps[0:4, 0, 0:BS]).then_inc(
                sOutD, 1
            )

    nc.compile()
    return nc


def _prep_inputs(inputs):
    import ml_dtypes

    bf = ml_dtypes.bfloat16
    p1 = np.asarray(inputs["patch1"], np.float32).reshape(B, C, HW)
    p2 = np.asarray(inputs["patch2"], np.float32).reshape(B, C, HW)
    X = np.empty((B, C, 98), bf)
    X[:, :, 0:49] = p1
    X[:, :, 49:98] = p2
    wst2 = _build_wst2(np.asarray(inputs["w_bbox"], np.float32)).astype(bf)
    wcv = np.asarray(wst2, bf)
    in_maps = []
    for c in range(N_CORES):
        Xc = X[c * BS : (c + 1) * BS]  # [BS, C, 98]
        ppv = np.ascontiguousarray(Xc.transpose(1, 0, 2))  # [C, BS, 98]
        in_maps.append({"pp": ppv, "wc": wcv})
    return in_maps


# device column b' holds sample 2*(b'%32) + b'//32
_COLS = np.arange(BS)
_SAMPLE_OF_COL = 2 * (_COLS % 32) + _COLS // 32


def _run(inputs, trace: bool = False):
    nc = build_nc()
    in_maps = _prep_inputs(inputs)
    bias = np.asarray(inputs["b_bbox"], np.float32)
    res = run_bass_kernel_spmd(
        nc, in_maps, core_ids=list(range(N_CORES)), trace=trace
    )
    parts = []
    for c in range(N_CORES):
        r = res.results[c]["out"]  # [100, BS]
        # fold the 4 chain partials (rows 32c+n) + bias
        folded = (
            r[0:4] + r[32:36] + r[64:68] + r[96:100]
        ).T + bias  # [BS, 4], permuted columns
        oc = np.empty((BS, 4), np.float32)
        oc[_SAMPLE_OF_COL] = folded
        parts.append(oc)
    out = np.concatenate(parts, axis=0).astype(np.float32)
    return out, res


def kernel(**inputs) -> np.ndarray:
    out, _ = _run(inputs, trace=False)
    return out


# revision 4
# speedup vs baseline: 1.1150x; 1.0121x over previous
"""Trainium2 Bass kernel for nn_CorrelationHead (8-core SPMD, data parallel over B).

Math: out[b,n] = sum_{yx,ij} (P2[b]^T P1[b])[yx,ij] * W3[yx,ij,n] + bias[n]

v5 (trace-driven):
  - Input: 5 chunk DMAs (16,16,16,8,8 samples) all on the sync HWDGE ring:
    one ring row keeps all 16 SDMA engines on one FIFO stream (v3 showed
    two rows halve effective bandwidth for the early chunks). Small last
    chunks cut the final sem-lag exposure.
  - Stage 1: one K=128 matmul per sample, alternating PE column strips
    (even samples -> psum partitions 0:49, odd -> 64:113) so LDWEIGHTS
    overlaps MATMUL. 8 samples per PSUM bank; all 64 fit -> no reuse sems.
  - Casts: evens on DVE, odds on ACT into acat2[128, 64, 49] (columns 0:32
    even samples rows 0:49 live, 32:64 odd rows 64:113 live; garbage
    regions zeroed once by gpsimd).
  - Stage 2: 49 K=113 matmuls, duplicated weights, 4 PE column strips.
  - Tail: the 4 chain partials are copied psum->SBUF f32 (DVE+ACT in
    parallel) and DMA'd out as [100, 64]; the 4-way fold + bias + column
    unpermute happen on the host. No final-DMA-completion wait: the
    end-of-block engine drain + fixed ~7.6us teardown dwarf the 26KB DMA.
"""

import numpy as np

import concourse.bass as bass
import concourse.mybir as mybir
from concourse import bacc
from concourse.bass_utils import run_bass_kernel_spmd

N_CORES = 8
B, C, HW = 512, 128, 49
BS = B // N_CORES    # 64 samples per core
CHUNKS = (8, 16, 16, 16, 8)
NG = 8               # psum bank groups
GS = BS // NG        # 8 samples per bank
PAD = 20

_F32 = mybir.dt.float32
_BF16 = mybir.dt.bfloat16


def _build_wst2(w_bbox: np.ndarray) -> np.ndarray:
    W3 = np.zeros((4, 49, 49), np.float32)
    for i in range(7):
        for j in range(7):
            for y in range(7):
                for x in range(7):
                    if (y - i) % 2 == 0 and (x - j) % 2 == 0:
                        p = (y - i + PAD) // 2
                        q = (x - j + PAD) // 2
                        W3[:, i * 7 + j, y * 7 + x] = w_bbox[
                            :, ((p * 21 + q) * 7 + i) * 7 + j
                        ]
    return np.ascontiguousarray(W3.transpose(2, 1, 0).reshape(49, 196))


def build_nc() -> bass.Bass:
    nc = bacc.Bacc("TRN2", target_bir_lowering=False, debug=False)
    pp = nc.dram_tensor("pp", [C, BS, 98], _BF16, kind="ExternalInput")
    wc = nc.dram_tensor("wc", [49, 196], _BF16, kind="ExternalInput")
    out = nc.dram_tensor("out", [100, BS], _F32, kind="ExternalOutput")

    from contextlib import ExitStack

    with ExitStack() as ctx:
        x_sb = ctx.enter_context(nc.sbuf_tensor("x_sb", [C, BS, 98], _BF16))
        acat2 = ctx.enter_context(nc.sbuf_tensor("acat2", [49, BS, HW], _BF16))
        wsb = ctx.enter_context(nc.sbuf_tensor("wsb", [49, 196], _BF16))
        outp = ctx.enter_context(nc.sbuf_tensor("outp", [100, BS], _F32))
        ps = ctx.enter_context(nc.psum_tensor("ps", [128, 8, 512], _F32))
        (sW, sMM, sCastD, sCastA, sS2, sOutD, sOutA, sDone) = (
            ctx.enter_context(nc.semaphore(nm))
            for nm in ("sW", "sMM", "sCastD", "sCastA", "sS2",
                       "sOutD", "sOutA", "sDone")
        )
        sIn = [ctx.enter_context(nc.semaphore(f"sIn{i}"))
               for i in range(len(CHUNKS))]
        block = ctx.enter_context(nc.Block())

        @block.sync
        def _(sync):
            o = 0
            for g, n in enumerate(CHUNKS):
                sync.dma_start(
                    out=x_sb[:, o : o + n, :], in_=pp[:, o : o + n, :]
                ).then_inc(sIn[g], 16)
                o += n
            sync.wait_ge(sOutD, 1)
            sync.wait_ge(sOutA, 1)
            sync.dma_start(out=out[:], in_=outp[:]).then_inc(sDone, 16)

        @block.scalar
        def _(scalar):
            for g in range(NG):  # odd-parity casts
                scalar.wait_ge(sMM, g + 1)
                nc.scalar.copy(
                    acat2[0:49, 32 + g * 4 : 32 + (g + 1) * 4, :],
                    ps[64:113, g, 196:392],
                ).then_inc(sCastA, 1)
            scalar.wait_ge(sS2, 3)
            nc.scalar.copy(outp[64:100, :], ps[64:100, 0, 0:BS]).then_inc(
                sOutA, 1
            )

        @block.gpsimd
        def _(gpsimd):
            gpsimd.dma_start(out=wsb[:], in_=wc[:]).then_inc(sW, 16)

        @block.tensor
        def _(tensor):
            bounds = []
            o = 0
            for n in CHUNKS:
                bounds.append(o)
                o += n
            for s in range(BS):
                if s in bounds:
                    tensor.wait_ge(sIn[bounds.index(s)], 16)
                g, idx, par = s // GS, (s % GS) // 2, s % 2
                col = par * 196 + idx * HW
                pbase = 64 * par
                mm = nc.tensor.matmul(
                    ps[pbase : pbase + 49, g, col : col + HW],
                    x_sb[:, s, 49:98],
                    x_sb[:, s, 0:49],
                    start=True,
                    stop=True,
                )
                if s % GS == GS - 1:
                    mm.then_inc(sMM, 1)
            tensor.wait_ge(sCastD, NG)
            tensor.wait_ge(sCastA, NG)
            tensor.wait_ge(sW, 16)
            for ij in range(HW):
                c = ij % 4
                mm = nc.tensor.matmul(
                    ps[32 * c : 32 * c + 4, 0, 0:BS],
                    wsb[:, ij * 4 : (ij + 1) * 4],
                    acat2[:, :, ij],
                    start=(ij < 4),
                    stop=(ij + 4 >= HW),
                    tile_position=(0, 32 * c),
                )
                if ij + 4 >= HW:
                    mm.then_inc(sS2, 1)

        @block.vector
        def _(vector):
            for g in range(NG):  # even-parity casts
                vector.wait_ge(sMM, g + 1)
                nc.vector.tensor_copy(
                    acat2[0:49, g * 4 : (g + 1) * 4, :],
                    ps[0:49, g, 0:196],
                ).then_inc(sCastD, 1)
            vector.wait_ge(sS2, 4)
            nc.vector.tensor_copy(outp[0:36, :], ps[0:36, 0, 0:BS]).then_inc(
                sOutD, 1
            )

    nc.compile()
    return nc


def _prep_inputs(inputs):
    import ml_dtypes

    bf = ml_dtypes.bfloat16
    p1 = np.asarray(inputs["patch1"], np.float32).reshape(B, C, HW)
    p2 = np.asarray(inputs["patch2"], np.float32).reshape(B, C, HW)
    X = np.empty((B, C, 98), bf)
    X[:, :, 0:49] = p1
    X[:, :, 49:98] = p2
    wst2 = _build_wst2(np.asarray(inputs["w_bbox"], np.float32)).astype(bf)
    wcv = np.asarray(wst2, bf)
    in_maps = []
    for c in range(N_CORES):
        Xc = X[c * BS : (c + 1) * BS]  # [BS, C, 98]
        ppv = np.ascontiguousarray(Xc.transpose(1, 0, 2))  # [C, BS, 98]
        in_maps.append({"pp": ppv, "wc": wcv})
    return in_maps


# device column b' holds sample 2*(b'%32) + b'//32
_COLS = np.arange(BS)
_SAMPLE_OF_COL = 2 * (_COLS % 32) + _COLS // 32


def _run(inputs, trace: bool = False):
    nc = build_nc()
    in_maps = _prep_inputs(inputs)
    bias = np.asarray(inputs["b_bbox"], np.float32)
    res = run_bass_kernel_spmd(
        nc, in_maps, core_ids=list(range(N_CORES)), trace=trace
    )
    parts = []
    for c in range(N_CORES):
        r = res.results[c]["out"]  # [100, BS]
        # fold the 4 chain partials (rows 32c+n) + bias
        folded = (
            r[0:4] + r[32:36] + r[64:68] + r[96:100]
        ).T + bias  # [BS, 4], permuted columns
        oc = np.empty((BS, 4), np.float32)
        oc[_SAMPLE_OF_COL] = folded
        parts.append(oc)
    out = np.concatenate(parts, axis=0).astype(np.float32)
    return out, res


def kernel(**inputs) -> np.ndarray:
    out, _ = _run(inputs, trace=False)
    return out


# revision 5
# speedup vs baseline: 1.1476x; 1.0292x over previous
"""Trainium2 Bass kernel for nn_CorrelationHead (8-core SPMD, data parallel over B).

Math: out[b,n] = sum_{yx,ij} (P2[b]^T P1[b])[yx,ij] * W3[yx,ij,n] + bias[n]

v5 (trace-driven):
  - Input: 5 chunk DMAs (16,16,16,8,8 samples) all on the sync HWDGE ring:
    one ring row keeps all 16 SDMA engines on one FIFO stream (v3 showed
    two rows halve effective bandwidth for the early chunks). Small last
    chunks cut the final sem-lag exposure.
  - Stage 1: one K=128 matmul per sample, alternating PE column strips
    (even samples -> psum partitions 0:49, odd -> 64:113) so LDWEIGHTS
    overlaps MATMUL. 8 samples per PSUM bank; all 64 fit -> no reuse sems.
  - Casts: evens on DVE, odds on ACT into acat2[128, 64, 49] (columns 0:32
    even samples rows 0:49 live, 32:64 odd rows 64:113 live; garbage
    regions zeroed once by gpsimd).
  - Stage 2: 49 K=113 matmuls, duplicated weights, 4 PE column strips.
  - Tail: the 4 chain partials are copied psum->SBUF f32 (DVE+ACT in
    parallel) and DMA'd out as [100, 64]; the 4-way fold + bias + column
    unpermute happen on the host. No final-DMA-completion wait: the
    end-of-block engine drain + fixed ~7.6us teardown dwarf the 26KB DMA.
"""

import numpy as np

import concourse.bass as bass
import concourse.mybir as mybir
from concourse import bacc
from concourse.bass_utils import run_bass_kernel_spmd

N_CORES = 8
B, C, HW = 512, 128, 49
BS = B // N_CORES    # 64 samples per core
CHUNKS = (8, 16, 16, 16, 8)
NG = 8               # psum bank groups
GS = BS // NG        # 8 samples per bank
PAD = 20

_F32 = mybir.dt.float32
_BF16 = mybir.dt.bfloat16


def _build_wst2(w_bbox: np.ndarray) -> np.ndarray:
    W3 = np.zeros((4, 49, 49), np.float32)
    for i in range(7):
        for j in range(7):
            for y in range(7):
                for x in range(7):
                    if (y - i) % 2 == 0 and (x - j) % 2 == 0:
                        p = (y - i + PAD) // 2
                        q = (x - j + PAD) // 2
                        W3[:, i * 7 + j, y * 7 + x] = w_bbox[
                            :, ((p * 21 + q) * 7 + i) * 7 + j
                        ]
    return np.ascontiguousarray(W3.transpose(2, 1, 0).reshape(49, 196))


def build_nc() -> bass.Bass:
    nc = bacc.Bacc("TRN2", target_bir_lowering=False, debug=False)
    pp = nc.dram_tensor("pp", [C, BS, 98], _BF16, kind="ExternalInput")
    wc = nc.dram_tensor("wc", [49, 196], _BF16, kind="ExternalInput")
    out = nc.dram_tensor("out", [100, BS], _F32, kind="ExternalOutput")

    from contextlib import ExitStack

    with ExitStack() as ctx:
        x_sb = ctx.enter_context(nc.sbuf_tensor("x_sb", [C, BS, 98], _BF16))
        acat2 = ctx.enter_context(nc.sbuf_tensor("acat2", [49, BS, HW], _BF16))
        wsb = ctx.enter_context(nc.sbuf_tensor("wsb", [49, 196], _BF16))
        outp = ctx.enter_context(nc.sbuf_tensor("outp", [100, BS], _F32))
        ps = ctx.enter_context(nc.psum_tensor("ps", [128, 8, 512], _F32))
        (sW, sMM, sCastD, sCastA, sS2, sOutD, sOutA, sDone) = (
            ctx.enter_context(nc.semaphore(nm))
            for nm in ("sW", "sMM", "sCastD", "sCastA", "sS2",
                       "sOutD", "sOutA", "sDone")
        )
        sIn = [ctx.enter_context(nc.semaphore(f"sIn{i}"))
               for i in range(len(CHUNKS))]
        block = ctx.enter_context(nc.Block())

        @block.sync
        def _(sync):
            o = CHUNKS[0]
            for g, n in list(enumerate(CHUNKS))[1:]:
                sync.dma_start(
                    out=x_sb[:, o : o + n, :], in_=pp[:, o : o + n, :]
                ).then_inc(sIn[g], 16)
                o += n
            sync.wait_ge(sOutD, 1)
            sync.wait_ge(sOutA, 1)
            sync.dma_start(out=out[:], in_=outp[:]).then_inc(sDone, 16)

        @block.scalar
        def _(scalar):
            scalar.dma_start(
                out=x_sb[:, 0 : CHUNKS[0], :], in_=pp[:, 0 : CHUNKS[0], :]
            ).then_inc(sIn[0], 16)
            for g in range(NG):  # odd-parity casts
                scalar.wait_ge(sMM, g + 1)
                nc.scalar.copy(
                    acat2[0:49, 32 + g * 4 : 32 + (g + 1) * 4, :],
                    ps[64:113, g, 196:392],
                ).then_inc(sCastA, 1)
            scalar.wait_ge(sS2, 3)
            nc.scalar.copy(outp[64:100, :], ps[64:100, 0, 0:BS]).then_inc(
                sOutA, 1
            )

        @block.gpsimd
        def _(gpsimd):
            gpsimd.dma_start(out=wsb[:], in_=wc[:]).then_inc(sW, 16)

        @block.tensor
        def _(tensor):
            bounds = []
            o = 0
            for n in CHUNKS:
                bounds.append(o)
                o += n
            for s in range(BS):
                if s in bounds:
                    tensor.wait_ge(sIn[bounds.index(s)], 16)
                g, idx, par = s // GS, (s % GS) // 2, s % 2
                col = par * 196 + idx * HW
                pbase = 64 * par
                mm = nc.tensor.matmul(
                    ps[pbase : pbase + 49, g, col : col + HW],
                    x_sb[:, s, 49:98],
                    x_sb[:, s, 0:49],
                    start=True,
                    stop=True,
                )
                if s % GS == GS - 1:
                    mm.then_inc(sMM, 1)
            tensor.wait_ge(sCastD, NG)
            tensor.wait_ge(sCastA, NG)
            tensor.wait_ge(sW, 16)
            for ij in range(HW):
                c = ij % 4
                mm = nc.tensor.matmul(
                    ps[32 * c : 32 * c + 4, 0, 0:BS],
                    wsb[:, ij * 4 : (ij + 1) * 4],
                    acat2[:, :, ij],
                    start=(ij < 4),
                    stop=(ij + 4 >= HW),
                    tile_position=(0, 32 * c),
                )
                if ij + 4 >= HW:
                    mm.then_inc(sS2, 1)

        @block.vector
        def _(vector):
            for g in range(NG):  # even-parity casts
                vector.wait_ge(sMM, g + 1)
                nc.vector.tensor_copy(
                    acat2[0:49, g * 4 : (g + 1) * 4, :],
                    ps[0:49, g, 0:196],
                ).then_inc(sCastD, 1)
            vector.wait_ge(sS2, 4)
            nc.vector.tensor_copy(outp[0:36, :], ps[0:36, 0, 0:BS]).then_inc(
                sOutD, 1
            )

    nc.compile()
    return nc


def _prep_inputs(inputs):
    import ml_dtypes

    bf = ml_dtypes.bfloat16
    p1 = np.asarray(inputs["patch1"], np.float32).reshape(B, C, HW)
    p2 = np.asarray(inputs["patch2"], np.float32).reshape(B, C, HW)
    X = np.empty((B, C, 98), bf)
    X[:, :, 0:49] = p1
    X[:, :, 49:98] = p2
    wst2 = _build_wst2(np.asarray(inputs["w_bbox"], np.float32)).astype(bf)
    wcv = np.asarray(wst2, bf)
    in_maps = []
    for c in range(N_CORES):
        Xc = X[c * BS : (c + 1) * BS]  # [BS, C, 98]
        ppv = np.ascontiguousarray(Xc.transpose(1, 0, 2))  # [C, BS, 98]
        in_maps.append({"pp": ppv, "wc": wcv})
    return in_maps


# device column b' holds sample 2*(b'%32) + b'//32
_COLS = np.arange(BS)
_SAMPLE_OF_COL = 2 * (_COLS % 32) + _COLS // 32


def _run(inputs, trace: bool = False):
    nc = build_nc()
    in_maps = _prep_inputs(inputs)
    bias = np.asarray(inputs["b_bbox"], np.float32)
    res = run_bass_kernel_spmd(
        nc, in_maps, core_ids=list(range(N_CORES)), trace=trace
    )
    parts = []
    for c in range(N_CORES):
        r = res.results[c]["out"]  # [100, BS]
        # fold the 4 chain partials (rows 32c+n) + bias
        folded = (
            r[0:4] + r[32:36] + r[64:68] + r[96:100]
        ).T + bias  # [BS, 4], permuted columns
        oc = np.empty((BS, 4), np.float32)
        oc[_SAMPLE_OF_COL] = folded
        parts.append(oc)
    out = np.concatenate(parts, axis=0).astype(np.float32)
    return out, res


def kernel(**inputs) -> np.ndarray:
    out, _ = _run(inputs, trace=False)
    return out
